# revision 1
# baseline (speedup 1.0000x reference)
"""DeepISP Trainium2 Bass kernel — 8-core SPMD, H-sharded with halo
redundancy, fold-2 row layout, bf16 matmuls with fp32 PSUM accumulation.

Sharding: core c owns output rows [64c, 64c+64). Local row l <-> global
64c - 12 + l, l in [0, 88). All full-res convs are computed per-core on
the halo-extended slice (no inter-layer communication); the high-level
path runs sharded down to pool2, then one AllGather replicates pool2 so
every core computes the tiny h3/gap/linear tail and the 3x10 color
matrix locally; the per-pixel quadratic Tform is applied to owned rows.

Fold-2 layout: activation buffers are [128 partitions, 44 super-rows,
514 cols] bf16 where partition p = parity*64 + channel, super-row s
holds image rows (2s, 2s+1), cols 0/513 are zero pads (conv W pad).
A 3x3 conv is, per output super-row j and kw in {0,1,2}:
  dense    [128x128] @ rhs = buf[:,   j,   kw:kw+512]
  q1 (64,0) [64x64]  @ rhs = buf[64:, j-1, kw:kw+512]  (kh'=0 -> even out)
  q3 (0,64) [64x64]  @ rhs = buf[:64, j+1, kw:kw+512]  (kh'=2 -> odd out)
accumulating into one PSUM bank [128, 512] = (even out row | odd row).
Matmuls are emitted weight-stationary over batches of 4 banks so the PE
pipelines at ~216 ns/slot (hardware-measured).
"""
import os
import sys

for _p in ("/opt/trn_rl_repo", "/root/.axon_site/_ro/trn_rl_repo"):
    if os.path.isdir(_p) and _p not in sys.path:
        sys.path.insert(0, _p)

import numpy as np
import ml_dtypes
from contextlib import ExitStack

import concourse.bass as bass
from concourse import bacc
import concourse.mybir as mybir
import concourse.tile as tile
from concourse.bass_utils import run_bass_kernel_spmd

bf16 = mybir.dt.bfloat16
f32 = mybir.dt.float32
AF = mybir.ActivationFunctionType
ALU = mybir.AluOpType
nbf = ml_dtypes.bfloat16

NCORES = 8
H = W = 512
HALO = 12          # local row 0 = global 64c-12
LR = 88
NSR = 44
SLAB = 514
BATCH = 4

R_I0 = (1, 42)
R_LL = [(1, 41), (2, 41), (2, 40), (3, 40)]
R_HL0 = (3, 39)
R_H1 = (2, 19)
ZT, ZB = 5, 38
OWN0 = 6           # owned output super-rows 6..37
NOWN = 32

# feats rows: X = [r,r,r,r, g,g,g, b,b, 1], Y = [r,g,b,1, g,b,1, b,1, 1]
# -> products [rr, rg, rb, r, gg, gb, g, bb, b, 1]  (reference order)
FEAT_PERM = list(range(10))

_cached = {}


def _batches(lo, hi, bsz=BATCH):
    out, j = [], lo
    while j <= hi:
        out.append(list(range(j, min(j + bsz, hi + 1))))
        j += bsz
    return out


def _ap(obj, d_part, extra_free, dims):
    """Custom AP anchored at (partition d_part, free elem offset) of an
    AP/tile view. dims = [[step,count],...] in elements (partition dim
    first, step in per-partition-element units for SBUF)."""
    a = obj[:] if hasattr(obj, "tile_context") or not isinstance(obj, bass.AP) else obj
    pstep = a.ap[0][0]
    return bass.AP(a.tensor, a.offset + d_part * pstep + extra_free,
                   [[dims[0][0] * pstep, dims[0][1]]] + list(dims[1:]))


# ---------------------------------------------------------------------------
# host-side weight packing
# ---------------------------------------------------------------------------

def _pack_dense(wfull, stride=1):
    out = np.zeros((128, 3 * 128), np.float32)
    for kw in range(3):
        blk = np.zeros((128, 128), np.float32)
        for a in range(2):
            for b in range(2):
                kh = a - stride * b + 1
                if 0 <= kh <= 2:
                    blk[a * 64:(a + 1) * 64, b * 64:(b + 1) * 64] = \
                        wfull[:, :, kh, kw].T
        out[:, kw * 128:(kw + 1) * 128] = blk
    return out


def _pack_quads(wfull):
    out = np.zeros((128, 3 * 64), np.float32)
    for kw in range(3):
        out[64:128, kw * 64:(kw + 1) * 64] = wfull[:, :, 0, kw].T
        out[0:64, kw * 64:(kw + 1) * 64] = wfull[:, :, 2, kw].T
    return out


def _pack_h1BC(w):
    B = np.zeros((128, 3 * 64), np.float32)
    C = np.zeros((128, 3 * 64), np.float32)
    for kw in range(3):
        B[64:128, kw * 64:(kw + 1) * 64] = w[:, :, 0, kw].T
        C[0:64, kw * 64:(kw + 1) * 64] = w[:, :, 1, kw].T
        C[64:128, kw * 64:(kw + 1) * 64] = w[:, :, 2, kw].T
    return B, C


def _pack_im2col_w(w):
    out = np.zeros((64, 9 * 64), np.float32)
    for t in range(9):
        kh, kw = divmod(t, 3)
        out[:, t * 64:(t + 1) * 64] = w[:, :, kh, kw].T
    return out


def _pack_weights(inp):
    pk = {}
    w27 = np.zeros((27, 64), np.float32)
    for kh in range(3):
        for kw in range(3):
            for ch in range(3):
                w27[(kh * 3 + kw) * 3 + ch, :] = inp["low0_w"][:, ch, kh, kw]
    pk["w_low0"] = w27

    main = []
    for i in range(4):
        wf = np.zeros((64, 64, 3, 3), np.float32)
        wf[:61, :61] = inp["ll_rh_w"][i]
        wf[61:, 61:] = inp["ll_lh_w"][i]
        main.append(np.concatenate([_pack_dense(wf), _pack_quads(wf)], 1))
    whl0 = np.zeros((64, 64, 3, 3), np.float32)
    whl0[:, :61] = inp["hl0_w"]
    main.append(np.concatenate([_pack_dense(whl0), _pack_quads(whl0)], 1))
    B, C = _pack_h1BC(inp["hl_w"][0])
    main.append(np.concatenate([_pack_dense(inp["hl_w"][0], stride=2), B, C], 1))
    pk["w_main"] = np.concatenate(main, 1)          # [128, 5*576 + 768]

    pk["w_h2h3"] = np.concatenate(
        [_pack_im2col_w(inp["hl_w"][1]), _pack_im2col_w(inp["hl_w"][2])], 1)
    pk["w_lin"] = (inp["lin_w"].T / 64.0).astype(np.float32)
    selL = np.zeros((30, 20), np.float32)
    for q in range(30):
        for p in range(20):
            if q % 10 == FEAT_PERM[p % 10]:
                selL[q, p] = 1.0
    pk["w_sel"] = selL
    cmask = np.zeros((30, 6), np.float32)
    for q in range(30):
        for n in range(6):
            if q // 10 == n % 3:
                cmask[q, n] = 1.0
    pk["cmask"] = cmask
    pmask = np.zeros((20, 6), np.float32)
    for p in range(20):
        for n in range(6):
            if p // 10 == n // 3:
                pmask[p, n] = 1.0
    pk["pmask"] = pmask

    bias = np.zeros((128, 9), np.float32)
    bias[0:64, 0] = bias[64:128, 0] = inp["low0_b"]
    for i in range(4):
        bb = np.concatenate([inp["ll_rh_b"][i], inp["ll_lh_b"][i]])
        bias[0:64, 1 + i] = bias[64:128, 1 + i] = bb
    bias[0:64, 5] = bias[64:128, 5] = inp["hl0_b"]
    bias[0:64, 6] = bias[64:128, 6] = inp["hl_b"][0]
    bias[0:64, 7] = inp["hl_b"][1]
    bias[0:64, 8] = inp["hl_b"][2]
    pk["bias"] = bias
    pk["lin_b"] = inp["lin_b"].reshape(30, 1).astype(np.float32)
    return pk


# ---------------------------------------------------------------------------
# device program
# ---------------------------------------------------------------------------

def _emit_fold_layer(nc, psum, src, dst, wts, bcol, bias_sb, mask_sb, rng, kind):
    wd = [wts[:, k * 128:(k + 1) * 128] for k in range(3)]
    wq = [wts[:, 384 + k * 64:384 + (k + 1) * 64] for k in range(3)]
    for batch in _batches(*rng):
        accs = [psum.tile([128, 512], f32, name=f"acc{i}", tag=f"b{i}")
                for i in range(len(batch))]
        for kw in range(3):
            for i, j in enumerate(batch):
                nc.tensor.matmul(accs[i][:], wd[kw], src[:, j, kw:kw + 512],
                                 start=(kw == 0), stop=False)
        for kw in range(3):
            for i, j in enumerate(batch):
                nc.tensor.matmul(accs[i][0:64, :], wq[kw][64:128, :],
                                 src[64:128, j - 1, kw:kw + 512],
                                 start=False, stop=False, tile_position=(64, 0))
            for i, j in enumerate(batch):
                nc.tensor.matmul(accs[i][64:128, :], wq[kw][0:64, :],
                                 src[0:64, j + 1, kw:kw + 512],
                                 start=False, stop=(kw == 2), tile_position=(0, 64))
        for i, j in enumerate(batch):
            acc, be = accs[i], bias_sb[:, bcol:bcol + 1]
            if kind == "ll":
                # tanh on [32:64]/[96:128]; DVE relus then overwrite the
                # rh channels (32..60 / 64..124) with the correct values.
                nc.scalar.activation(dst[32:64, j, 1:513], acc[32:64, :],
                                     AF.Tanh, bias=be[32:64])
                nc.scalar.activation(dst[96:128, j, 1:513], acc[96:128, :],
                                     AF.Tanh, bias=be[96:128])
                nc.vector.tensor_scalar(dst[0:61, j, 1:513], acc[0:61, :],
                                        be[0:61], 0.0, ALU.add, ALU.max)
                nc.vector.tensor_scalar(dst[64:125, j, 1:513], acc[64:125, :],
                                        be[64:125], 0.0, ALU.add, ALU.max)
            else:
                nc.vector.tensor_scalar(dst[0:64, j, 1:513], acc[0:64, :],
                                        be[0:64], None, ALU.add)
                nc.scalar.activation(dst[64:128, j, 1:513], acc[64:128, :],
                                     AF.Identity, bias=be[64:128])
            if j <= ZT or j >= ZB:
                nc.vector.tensor_scalar_mul(dst[:, j, 1:513], dst[:, j, 1:513],
                                            mask_sb[:, j:j + 1])


def _emit_feats(nc, bufA, xT, yT, featsT, half):
    """Build feats for one 16-sr half (half in {0,1}) of the owned region,
    in two 8-sr sub-chunks, into featsT [20, 16*512]."""
    if half == 0:
        nc.gpsimd.memset(xT[:], 1.0)
        nc.gpsimd.memset(yT[:], 1.0)
    for sub in range(2):
        s0 = OWN0 + half * 16 + sub * 8
        doff = 0
        XRUNS = [(0, 4), (4, 3), (7, 2)]          # r,g,b -> X row runs
        YRUNS = [[0], [1, 4], [2, 5, 7]]          # r,g,b -> Y rows
        for par in range(2):
            ba = bufA[:]
            for ci in range(3):
                sp = 61 + ci + 64 * par
                soff = ba.offset + sp * ba.ap[0][0] + s0 * SLAB
                p0, n = XRUNS[ci]
                srcap = bass.AP(ba.tensor, soff,
                                [[ba.ap[0][0], 1], [0, n], [1, 8 * SLAB]])
                dst = _ap(xT, 10 * par + p0, doff, [[1, n], [1, 8 * SLAB]])
                nc.sync.dma_start(dst, srcap)
                for yp in YRUNS[ci]:
                    srcy = bass.AP(ba.tensor, soff,
                                   [[ba.ap[0][0], 1], [1, 8 * SLAB]])
                    dsty = _ap(yT, 10 * par + yp, doff, [[1, 1], [1, 8 * SLAB]])
                    nc.sync.dma_start(dsty, srcy)
        nc.vector.tensor_mul(featsT[:, sub * 8 * SLAB:(sub + 1) * 8 * SLAB],
                             xT[:], yT[:])


def _build_program(debug=False, upto=None):
    nc = bacc.Bacc("TRN2", target_bir_lowering=False, debug=False,
                   num_devices=NCORES)

    x_in = nc.dram_tensor("x", [3, LR, SLAB], bf16, kind="ExternalInput")
    wmain_in = nc.dram_tensor("w_main", [128, 5 * 576 + 768], bf16,
                              kind="ExternalInput")
    wlow0_in = nc.dram_tensor("w_low0", [27, 64], bf16, kind="ExternalInput")
    wh23_in = nc.dram_tensor("w_h2h3", [64, 1152], bf16, kind="ExternalInput")
    wsel_in = nc.dram_tensor("w_sel", [30, 20], bf16, kind="ExternalInput")
    cmask_in = nc.dram_tensor("cmask", [30, 6], f32, kind="ExternalInput")
    pmask_in = nc.dram_tensor("pmask", [20, 6], f32, kind="ExternalInput")
    wlin_in = nc.dram_tensor("w_lin", [64, 30], f32, kind="ExternalInput")
    bias_in = nc.dram_tensor("bias", [128, 9], f32, kind="ExternalInput")
    linb_in = nc.dram_tensor("lin_b", [30, 1], f32, kind="ExternalInput")
    maskI_in = nc.dram_tensor("mask_i", [128, NSR], f32, kind="ExternalInput")
    maskh1_in = nc.dram_tensor("mask_h1", [128, 20], f32, kind="ExternalInput")

    out_d = nc.dram_tensor("out", [3, 64, W], f32, kind="ExternalOutput")

    h1_dram = nc.dram_tensor("h1_dram", [128, 18 * 256], bf16)
    wp_dram = nc.dram_tensor("wp_dram", [30], f32)
    cc_in = nc.dram_tensor("cc_in", [64 * 4 * 32], bf16)
    cc_gath = nc.dram_tensor("cc_gath", [NCORES * 64 * 4 * 32], bf16,
                             addr_space="Shared")
    dbg = {}
    if debug:
        for nm, shp, dt in [("i0", [128, NSR * SLAB], bf16),
                            ("i1", [128, NSR * SLAB], bf16),
                            ("i2", [128, NSR * SLAB], bf16),
                            ("i3", [128, NSR * SLAB], bf16),
                            ("i4", [128, NSR * SLAB], bf16),
                            ("hl0", [128, NSR * SLAB], bf16),
                            ("h1", [128, 18 * 256], bf16),
                            ("pool1", [64, 20 * 130], bf16),
                            ("pool2", [64, 4 * 32], bf16),
                            ("wp", [30, 1], f32)]:
            dbg[nm] = nc.dram_tensor("dbg_" + nm, shp, dt, kind="ExternalOutput")

    with tile.TileContext(nc) as tc, ExitStack() as ctx:
        pers = ctx.enter_context(tc.tile_pool(name="pers", bufs=1))
        psum = ctx.enter_context(tc.tile_pool(name="psum", bufs=2, space="PSUM"))

        w_main = pers.tile([128, 5 * 576 + 768], bf16)
        nc.sync.dma_start(w_main[:], wmain_in[:])
        w_low0 = pers.tile([27, 64], bf16)
        nc.sync.dma_start(w_low0[:], wlow0_in[:])
        w_h2h3 = pers.tile([64, 1152], bf16)
        w_sel = pers.tile([30, 20], bf16)
        nc.sync.dma_start(w_sel[:], wsel_in[:])
        cmask_sb = pers.tile([30, 6], f32)
        nc.sync.dma_start(cmask_sb[:], cmask_in[:])
        pmask_sb = pers.tile([20, 6], f32)
        nc.sync.dma_start(pmask_sb[:], pmask_in[:])
        nc.sync.dma_start(w_h2h3[:], wh23_in[:])
        w_lin = pers.tile([64, 30], f32)
        nc.sync.dma_start(w_lin[:], wlin_in[:])
        bias_sb = pers.tile([128, 9], f32)
        nc.sync.dma_start(bias_sb[:], bias_in[:])
        linb_sb = pers.tile([30, 1], f32)
        nc.sync.dma_start(linb_sb[:], linb_in[:])
        mask_sb = pers.tile([128, NSR], f32)
        nc.sync.dma_start(mask_sb[:], maskI_in[:])
        maskh1_sb = pers.tile([128, 20], f32)
        nc.sync.dma_start(maskh1_sb[:], maskh1_in[:])

        bufA = pers.tile([128, NSR, SLAB], bf16)
        bufB = pers.tile([128, NSR, SLAB], bf16)
        nc.gpsimd.memset(bufA[:], 0.0)
        nc.gpsimd.memset(bufB[:], 0.0)

        # ---- low0 via im2col: out rows 2..85 = slots 0..83, chunks of 22
        with tc.tile_pool(name="imcp", bufs=2) as imcp:
            for c0 in range(0, 84, 22):
                ns = min(22, 84 - c0)
                imc = imcp.tile([27, 22 * 512], bf16, name="imc", tag="imc")
                for kh in range(3):
                    for kw in range(3):
                        t = kh * 3 + kw
                        src = bass.AP(x_in[:].tensor,
                                      (c0 + 1 + kh) * SLAB + kw,
                                      [[LR * SLAB, 3], [SLAB, ns], [1, 512]])
                        nc.sync.dma_start(imc[3 * t:3 * t + 3, 0:ns * 512], src)
                srs = [j for j in range(R_I0[0], R_I0[1] + 1)
                       if c0 <= 2 * j - 2 and 2 * j - 1 < c0 + ns]
                for bt in _batches(srs[0], srs[-1]):
                    accs = [psum.tile([128, 512], f32, name=f"acc{i}",
                                      tag=f"b{i}") for i in range(len(bt))]
                    for i, j in enumerate(bt):
                        se = 2 * j - 2 - c0
                        nc.tensor.matmul(accs[i][0:64, :], w_low0[:],
                                         imc[:, se * 512:(se + 1) * 512],
                                         start=True, stop=True,
                                         tile_position=(0, 0))
                        nc.tensor.matmul(accs[i][64:128, :], w_low0[:],
                                         imc[:, (se + 1) * 512:(se + 2) * 512],
                                         start=True, stop=True,
                                         tile_position=(0, 64))
                    for i, j in enumerate(bt):
                        be = bias_sb[:, 0:1]
                        nc.vector.tensor_scalar(bufA[0:64, j, 1:513],
                                                accs[i][0:64, :], be[0:64],
                                                None, ALU.add)
                        nc.scalar.activation(bufA[64:128, j, 1:513],
                                             accs[i][64:128, :], AF.Identity,
                                             bias=be[64:128])
                        if j <= ZT or j >= ZB:
                            nc.vector.tensor_scalar_mul(
                                bufA[:, j, 1:513], bufA[:, j, 1:513],
                                mask_sb[:, j:j + 1])

        if debug:
            nc.sync.dma_start(dbg["i0"][:],
                              bufA[:].rearrange("p a b -> p (a b)"))
        _stop_after_low0 = (upto == "i0")

        if _stop_after_low0:
            pass
        else:
         with tc.tile_pool(name="tform", bufs=1) as tfp:
            featsT = tfp.tile([20, 16 * SLAB], bf16)
            xT = tfp.tile([20, 8 * SLAB], bf16)
            yT = tfp.tile([20, 8 * SLAB], bf16)
            outstage = tfp.tile([6, 8 * 512], f32)
            wm20 = tfp.tile([20, 6], bf16)

            # ---- ll layers + hl0 ----
            bufs = [bufA, bufB]
            for i in range(4):
                _emit_fold_layer(nc, psum, bufs[i % 2], bufs[(i + 1) % 2],
                                 w_main[:, i * 576:(i + 1) * 576], 1 + i,
                                 bias_sb, mask_sb, R_LL[i], "ll")
                if debug and i < 3:
                    nc.sync.dma_start(
                        dbg[f"i{i + 1}"][:],
                        bufs[(i + 1) % 2][:].rearrange("p a b -> p (a b)"))
            _emit_fold_layer(nc, psum, bufA, bufB, w_main[:, 4 * 576:5 * 576],
                             5, bias_sb, mask_sb, R_HL0, "copy")
            if debug:
                nc.sync.dma_start(dbg["i4"][:],
                                  bufA[:].rearrange("p a b -> p (a b)"))
                nc.sync.dma_start(dbg["hl0"][:],
                                  bufB[:].rearrange("p a b -> p (a b)"))

            with tc.tile_pool(name="hlp", bufs=1) as hlp:
                # ---- h1 (stride-2 fold conv from bufB) ----
                wh1 = w_main[:, 5 * 576:]
                wA = [wh1[:, k * 128:(k + 1) * 128] for k in range(3)]
                wB = [wh1[:, 384 + k * 64:384 + (k + 1) * 64] for k in range(3)]
                wC = [wh1[:, 576 + k * 64:576 + (k + 1) * 64] for k in range(3)]
                h1fold = hlp.tile([128, 18 * 256], bf16)
                for batch in _batches(*R_H1):
                    accs = [psum.tile([128, 256], f32, name=f"acc{i}",
                                      tag=f"b{i}") for i in range(len(batch))]
                    for kw in range(3):
                        for i, m in enumerate(batch):
                            nc.tensor.matmul(accs[i][:], wA[kw],
                                             bufB[:, 2 * m, kw:kw + 512:2],
                                             start=(kw == 0), stop=False)
                    for kw in range(3):
                        for i, m in enumerate(batch):
                            nc.tensor.matmul(accs[i][0:64, :], wB[kw][64:128, :],
                                             bufB[64:128, 2 * m - 1, kw:kw + 512:2],
                                             start=False, stop=False,
                                             tile_position=(64, 0))
                        for i, m in enumerate(batch):
                            nc.tensor.matmul(accs[i][64:128, :], wC[kw][:],
                                             bufB[:, 2 * m + 1, kw:kw + 512:2],
                                             start=False, stop=(kw == 2),
                                             tile_position=(0, 64))
                    for i, m in enumerate(batch):
                        be = bias_sb[:, 6:7]
                        sl = h1fold[:, (m - 2) * 256:(m - 1) * 256]
                        nc.vector.tensor_scalar(sl[0:64, :], accs[i][0:64, :],
                                                be[0:64], 0.0, ALU.add, ALU.max)
                        nc.scalar.activation(sl[64:128, :], accs[i][64:128, :],
                                             AF.Relu, bias=be[64:128])
                        if m in (2, 3, 18, 19):
                            nc.vector.tensor_scalar_mul(sl[:], sl[:],
                                                        maskh1_sb[:, m:m + 1])
                nc.sync.dma_start(h1_dram[:], h1fold[:])
                if debug:
                    nc.sync.dma_start(dbg["h1"][:], h1fold[:])

                # unfold h1 -> h1flat [64, 36, 256] (2 DMAs, per parity)
                h1flat = hlp.tile([64, 36, 256], bf16)
                for par in range(2):
                    src = bass.AP(h1_dram[:].tensor, par * 64 * 18 * 256,
                                  [[18 * 256, 64], [256, 18], [1, 256]])
                    dst = _ap(h1flat, 0, par * 256,
                              [[1, 64], [512, 18], [1, 256]])
                    nc.sync.dma_start(dst, src)

                # pool1 [64, 20, 130]
                pool1 = hlp.tile([64, 20, 130], bf16)
                nc.gpsimd.memset(pool1[:], 0.0)
                tmp1 = hlp.tile([64, 36, 128], bf16)
                nc.vector.tensor_max(tmp1[:], h1flat[:, :, 0:256:2],
                                     h1flat[:, :, 1:256:2])
                nc.vector.tensor_max(pool1[:, 2:20, 1:129],
                                     tmp1[:, 0:36:2, :], tmp1[:, 1:36:2, :])
                if debug:
                    nc.sync.dma_start(dbg["pool1"][:],
                                      pool1[:].rearrange("p a b -> p (a b)"))

                # ---- h2 via im2col (DVE gather, 9 taps, K=64) ----
                imc2 = hlp.tile([64, 9 * 512], bf16)
                for t in range(9):
                    kh, kw = divmod(t, 3)
                    src = _ap(pool1, 0, (2 + kh) * 130 + kw,
                              [[1, 64], [2 * 130, 8], [2, 64]])
                    nc.vector.tensor_copy(imc2[:, t * 512:(t + 1) * 512], src)
                acc2 = psum.tile([64, 512], f32, name="acc0", tag="b0")
                for t in range(9):
                    nc.tensor.matmul(acc2[:], w_h2h3[:, t * 64:(t + 1) * 64],
                                     imc2[:, t * 512:(t + 1) * 512],
                                     start=(t == 0), stop=(t == 8))
                h2sb = hlp.tile([64, 8, 64], bf16)
                nc.scalar.activation(h2sb[:].rearrange("p a b -> p (a b)"),
                                     acc2[:], AF.Relu, bias=bias_sb[0:64, 7:8])

                # pool2 -> cc_in
                tmp2 = hlp.tile([64, 8, 32], bf16)
                nc.vector.tensor_max(tmp2[:], h2sb[:, :, 0:64:2],
                                     h2sb[:, :, 1:64:2])
                pool2 = hlp.tile([64, 4, 32], bf16)
                nc.vector.tensor_max(pool2[:], tmp2[:, 0:8:2, :],
                                     tmp2[:, 1:8:2, :])
                nc.sync.dma_start(cc_in[:],
                                  pool2[:].rearrange("p a b -> p (a b)"))
                if debug:
                    nc.sync.dma_start(dbg["pool2"][:],
                                      pool2[:].rearrange("p a b -> p (a b)"))

                # ---- AllGather pool2 ----
                with tc.tile_critical():
                    cc_sem = nc.alloc_semaphore("cc_sem")
                    nc.gpsimd.collective_compute(
                        "AllGather", ALU.bypass,
                        replica_groups=[list(range(NCORES))],
                        ins=[cc_in[:]], outs=[cc_gath[:]],
                    ).then_inc(cc_sem)
                    nc.gpsimd.wait_ge(cc_sem, 1)

                # feats half 0 overlaps the collective
                _emit_feats(nc, bufA, xT, yT, featsT, 0)

                # ---- h3 tail (replicated) ----
                p2f = hlp.tile([64, 34, 34], bf16)
                nc.gpsimd.memset(p2f[:], 0.0)
                for q in range(NCORES):
                    src = bass.AP(cc_gath[:].tensor, q * 64 * 4 * 32,
                                  [[4 * 32, 64], [32, 4], [1, 32]])
                    nc.sync.dma_start(p2f[:, 1 + 4 * q:5 + 4 * q, 1:33], src)
                imc3 = hlp.tile([64, 9 * 256], bf16)
                for t in range(9):
                    kh, kw = divmod(t, 3)
                    src = _ap(p2f, 0, kh * 34 + kw,
                              [[1, 64], [2 * 34, 16], [2, 16]])
                    nc.vector.tensor_copy(imc3[:, t * 256:(t + 1) * 256], src)
                acc3 = psum.tile([64, 256], f32, name="acc1", tag="b1")
                for t in range(9):
                    nc.tensor.matmul(acc3[:],
                                     w_h2h3[:, 576 + t * 64:576 + (t + 1) * 64],
                                     imc3[:, t * 256:(t + 1) * 256],
                                     start=(t == 0), stop=(t == 8))
                h3sb = hlp.tile([64, 16, 16], bf16)
                nc.scalar.activation(h3sb[:].rearrange("p a b -> p (a b)"),
                                     acc3[:], AF.Relu, bias=bias_sb[0:64, 8:9])
                tmp3 = hlp.tile([64, 16, 8], bf16)
                nc.vector.tensor_max(tmp3[:], h3sb[:, :, 0:16:2],
                                     h3sb[:, :, 1:16:2])
                h3p = hlp.tile([64, 8, 8], f32)
                nc.vector.tensor_max(h3p[:], tmp3[:, 0:16:2, :],
                                     tmp3[:, 1:16:2, :])
                gsum = hlp.tile([64, 1], f32)
                nc.vector.reduce_sum(gsum[:],
                                     h3p[:].rearrange("p a b -> p (a b)"),
                                     axis=mybir.AxisListType.X)
                accW = psum.tile([30, 1], f32, name="acc2", tag="b2")
                nc.tensor.matmul(accW[:], w_lin[:], gsum[:],
                                 start=True, stop=True)
                wp_sb = hlp.tile([30, 1], f32)
                nc.scalar.activation(wp_sb[:], accW[:], AF.Identity,
                                     bias=linb_sb[:])
                if debug:
                    nc.sync.dma_start(dbg["wp"][:], wp_sb[:])
                wpR = hlp.tile([30, 6], bf16)
                nc.vector.tensor_scalar_mul(wpR[:], cmask_sb[:], wp_sb[:])
                accM = psum.tile([20, 6], f32, name="acc3", tag="b3")
                nc.tensor.matmul(accM[:], w_sel[:], wpR[:],
                                 start=True, stop=True)
                nc.vector.tensor_tensor(wm20[:], accM[:], pmask_sb[:],
                                        ALU.mult)

                # Tform matmuls for half 0 (inside hlp is fine)
                _emit_tform_half(nc, psum, wm20, featsT, outstage, out_d, 0)

            # ---- half 1 ----
            _emit_feats(nc, bufA, xT, yT, featsT, 1)
            _emit_tform_half(nc, psum, wm20, featsT, outstage, out_d, 1)

    nc.compile()
    return nc


def _emit_tform_half(nc, psum, wm20, featsT, outstage, out_d, half):
    for sub in range(2):
        srs = [OWN0 + half * 16 + sub * 8 + i for i in range(8)]
        ost = outstage  # [6, 8*512] f32, bufs=2 rotation via tag reuse
        for bi, bt in enumerate([srs[0:4], srs[4:8]]):
            accs = [psum.tile([6, 512], f32, name=f"acc{i}", tag=f"b{i}")
                    for i in range(len(bt))]
            for i, j in enumerate(bt):
                fo = (j - OWN0 - half * 16) * SLAB + 1
                nc.tensor.matmul(accs[i][:], wm20[:], featsT[:, fo:fo + 512],
                                 start=True, stop=True)
            for i, j in enumerate(bt):
                so = (j - srs[0]) * 512
                nc.scalar.activation(ost[:, so:so + 512], accs[i][:], AF.Copy)
        # DMA this 8-sr block to out: rows 2*(srs[0]-OWN0) ..
        r0 = 2 * (srs[0] - OWN0)
        for par in range(2):
            sap = _ap(ost, par * 3, 0, [[1, 3], [512, 8], [1, 512]])
            dap = bass.AP(out_d[:].tensor, (r0 + par) * 512,
                          [[64 * 512, 3], [2 * 512, 8], [1, 512]])
            nc.sync.dma_start(dap, sap)


# ---------------------------------------------------------------------------
# host entry
# ---------------------------------------------------------------------------

def kernel(**inputs):
    inp = {k: np.asarray(v) for k, v in inputs.items()}
    debug = bool(_cached.get("debug", False))
    key = ("nc", debug)
    if key not in _cached:
        _cached[key] = _build_program(debug=debug)
    nc = _cached[key]

    pk = _pack_weights(inp)
    x = np.asarray(inp["x"], np.float32)[0]

    shared = {
        "w_main": pk["w_main"].astype(nbf),
        "w_low0": pk["w_low0"].astype(nbf),
        "w_h2h3": pk["w_h2h3"].astype(nbf),
        "w_sel": pk["w_sel"].astype(nbf),
        "cmask": pk["cmask"],
        "pmask": pk["pmask"],
        "w_lin": pk["w_lin"],
        "bias": pk["bias"],
        "lin_b": pk["lin_b"],
    }
    in_maps = []
    par_col = (np.arange(128) // 64)[:, None]
    for c in range(NCORES):
        g0 = 64 * c - HALO
        xs = np.zeros((3, LR, SLAB), np.float32)
        lo, hi = max(0, -g0), min(LR, H - g0)
        xs[:, lo:hi, 1:513] = x[:, g0 + lo:g0 + hi, :]
        gI = g0 + 2 * np.arange(NSR)[None, :] + par_col
        maskI = ((gI >= 0) & (gI < H)).astype(np.float32)
        gh = 32 * c - 6 + 2 * np.arange(20)[None, :] + par_col
        maskh1 = ((gh >= 0) & (gh < 256)).astype(np.float32)
        im = dict(shared)
        im["x"] = xs.astype(nbf)
        im["mask_i"] = maskI
        im["mask_h1"] = maskh1
        in_maps.append(im)

    res = run_bass_kernel_spmd(nc, in_maps, list(range(NCORES)))
    _cached["last_results"] = res
    out = np.concatenate([res.results[c]["out"] for c in range(NCORES)], axis=1)
    return out[None].astype(np.float32)



# revision 5
# speedup vs baseline: 1.1282x; 1.1282x over previous
"""DeepISP Trainium2 Bass kernel — 8-core SPMD, H-sharded with halo
redundancy, fold-2 row layout, bf16 matmuls with fp32 PSUM accumulation.

Sharding: core c owns output rows [64c, 64c+64). Local row l <-> global
64c - 12 + l, l in [0, 88). All full-res convs are computed per-core on
the halo-extended slice (no inter-layer communication); the high-level
path runs sharded down to pool2, then one AllGather replicates pool2 so
every core computes the tiny h3/gap/linear tail and the 3x10 color
matrix locally; the per-pixel quadratic Tform is applied to owned rows.

Fold-2 layout: activation buffers are [128 partitions, 44 super-rows,
514 cols] bf16 where partition p = parity*64 + channel, super-row s
holds image rows (2s, 2s+1), cols 0/513 are zero pads (conv W pad).
A 3x3 conv is, per output super-row j and kw in {0,1,2}:
  dense    [128x128] @ rhs = buf[:,   j,   kw:kw+512]
  q1 (64,0) [64x64]  @ rhs = buf[64:, j-1, kw:kw+512]  (kh'=0 -> even out)
  q3 (0,64) [64x64]  @ rhs = buf[:64, j+1, kw:kw+512]  (kh'=2 -> odd out)
accumulating into one PSUM bank [128, 512] = (even out row | odd row).

The 3-channel tanh (lh) subpath is fully independent of the 61-channel
relu (rh) path, so it runs separately in a row-packed layout L
(partition = row_off*3 + ch) where each 3x3 conv layer is 3 banded
block-Toeplitz matmuls [3*in_rows -> 3*out_rows] plus ONE tanh per
block — instead of per-super-row tanh activations.  The rh epilogue
(bias+relu PSUM->SBUF drain) is a single full-width op per super-row,
rotated across the DVE / Act / Pool engines.  Edge super-rows fold the
image-boundary mask into the Act op via per-partition scale/bias.
"""
import os
import sys

for _p in ("/opt/trn_rl_repo", "/root/.axon_site/_ro/trn_rl_repo"):
    if os.path.isdir(_p) and _p not in sys.path:
        sys.path.insert(0, _p)

import numpy as np
import ml_dtypes
from contextlib import ExitStack

import concourse.bass as bass
from concourse import bacc
import concourse.mybir as mybir
import concourse.tile as tile
from concourse.bass_utils import run_bass_kernel_spmd

bf16 = mybir.dt.bfloat16
f32 = mybir.dt.float32
AF = mybir.ActivationFunctionType
ALU = mybir.AluOpType
nbf = ml_dtypes.bfloat16

NCORES = 8
H = W = 512
HALO = 12          # local row 0 = global 64c-12
LR = 88
NSR = 44
SLAB = 514
BATCH = 4

R_I0 = (1, 42)
R_LL = [(1, 41), (2, 41), (2, 40), (3, 40)]
R_HL0 = (3, 39)
R_H1 = (2, 19)
ZT, ZB = 5, 38
OWN0 = 6           # owned output super-rows 6..37
NOWN = 32

EDGE = [1, 2, 3, 4, 5, 38, 39, 40, 41, 42]   # edge super-rows (masked)
H1EDGE = [2, 3, 18, 19]

# lh (tanh) pipeline blocks: block j covers final rows [A+4, A+4+out4)
LH_A = [8, 30, 52]
LH_IN0 = [30, 30, 28]

_cached = {}


def _batches(lo, hi, bsz=BATCH):
    out, j = [], lo
    while j <= hi:
        out.append(list(range(j, min(j + bsz, hi + 1))))
        j += bsz
    return out


def _ap(obj, d_part, extra_free, dims):
    """Custom AP anchored at (partition d_part, free elem offset) of an
    AP/tile view. dims = [[step,count],...] in elements (partition dim
    first, step in per-partition-element units for SBUF)."""
    a = obj[:] if hasattr(obj, "tile_context") or not isinstance(obj, bass.AP) else obj
    pstep = a.ap[0][0]
    return bass.AP(a.tensor, a.offset + d_part * pstep + extra_free,
                   [[dims[0][0] * pstep, dims[0][1]]] + list(dims[1:]))


# ---------------------------------------------------------------------------
# host-side weight packing
# ---------------------------------------------------------------------------

def _pack_dense(wfull, stride=1):
    out = np.zeros((128, 3 * 128), np.float32)
    for kw in range(3):
        blk = np.zeros((128, 128), np.float32)
        for a in range(2):
            for b in range(2):
                kh = a - stride * b + 1
                if 0 <= kh <= 2:
                    blk[a * 64:(a + 1) * 64, b * 64:(b + 1) * 64] = \
                        wfull[:, :, kh, kw].T
        out[:, kw * 128:(kw + 1) * 128] = blk
    return out


def _pack_quads(wfull):
    out = np.zeros((128, 3 * 64), np.float32)
    for kw in range(3):
        out[64:128, kw * 64:(kw + 1) * 64] = wfull[:, :, 0, kw].T
        out[0:64, kw * 64:(kw + 1) * 64] = wfull[:, :, 2, kw].T
    return out


def _pack_h1BC(w):
    B = np.zeros((128, 3 * 64), np.float32)
    C = np.zeros((128, 3 * 64), np.float32)
    for kw in range(3):
        B[64:128, kw * 64:(kw + 1) * 64] = w[:, :, 0, kw].T
        C[0:64, kw * 64:(kw + 1) * 64] = w[:, :, 1, kw].T
        C[64:128, kw * 64:(kw + 1) * 64] = w[:, :, 2, kw].T
    return B, C


def _pack_im2col_w(w):
    out = np.zeros((64, 9 * 64), np.float32)
    for t in range(9):
        kh, kw = divmod(t, 3)
        out[:, t * 64:(t + 1) * 64] = w[:, :, kh, kw].T
    return out


def _pack_weights(inp):
    pk = {}
    w27 = np.zeros((27, 64), np.float32)
    for kh in range(3):
        for kw in range(3):
            for ch in range(3):
                w27[(kh * 3 + kw) * 3 + ch, :] = inp["low0_w"][:, ch, kh, kw]
    pk["w_low0"] = w27

    main = []
    for i in range(4):
        wf = np.zeros((64, 64, 3, 3), np.float32)
        wf[:61, :61] = inp["ll_rh_w"][i]          # lh block stays ZERO
        main.append(np.concatenate([_pack_dense(wf), _pack_quads(wf)], 1))
    whl0 = np.zeros((64, 64, 3, 3), np.float32)
    whl0[:, :61] = inp["hl0_w"]
    main.append(np.concatenate([_pack_dense(whl0), _pack_quads(whl0)], 1))
    B, C = _pack_h1BC(inp["hl_w"][0])
    main.append(np.concatenate([_pack_dense(inp["hl_w"][0], stride=2), B, C], 1))
    pk["w_main"] = np.concatenate(main, 1)          # [128, 5*576 + 768]

    pk["w_h2h3"] = np.concatenate(
        [_pack_im2col_w(inp["hl_w"][1]), _pack_im2col_w(inp["hl_w"][2])], 1)
    pk["w_lin"] = (inp["lin_w"].T / 64.0).astype(np.float32)
    selL = np.zeros((30, 20), np.float32)
    for q in range(30):
        for p in range(20):
            if q % 10 == p % 10:
                selL[q, p] = 1.0
    pk["w_sel"] = selL
    cmask = np.zeros((30, 6), np.float32)
    for q in range(30):
        for n in range(6):
            if q // 10 == n % 3:
                cmask[q, n] = 1.0
    pk["cmask"] = cmask
    pmask = np.zeros((20, 6), np.float32)
    for p in range(20):
        for n in range(6):
            if p // 10 == n // 3:
                pmask[p, n] = 1.0
    pk["pmask"] = pmask

    bias = np.zeros((128, 9), np.float32)
    bias[0:64, 0] = bias[64:128, 0] = inp["low0_b"]
    for i in range(4):
        bias[0:61, 1 + i] = bias[64:125, 1 + i] = inp["ll_rh_b"][i]
    bias[0:64, 5] = bias[64:128, 5] = inp["hl0_b"]
    bias[0:64, 6] = bias[64:128, 6] = inp["hl_b"][0]
    bias[0:64, 7] = inp["hl_b"][1]
    bias[0:64, 8] = inp["hl_b"][2]
    pk["bias"] = bias
    pk["lin_b"] = inp["lin_b"].reshape(30, 1).astype(np.float32)
    return pk


def _pack_lh_core(inp, core):
    """Per-core banded Toeplitz weights + masked bias for the lh path."""
    def in_img(l):
        g = 64 * core - HALO + l
        return 0 <= g < H

    w = np.zeros((90, 36 * 84), np.float32)
    b = np.zeros((84, 12), np.float32)
    for j in range(3):
        for i in range(4):
            in_r = LH_IN0[j] - 2 * i
            out_r = in_r - 2
            base = ((j * 4 + i) * 3) * 84
            for kw in range(3):
                for o in range(out_r):
                    row = LH_A[j] + i + 1 + o
                    m = 1.0 if in_img(row) else 0.0
                    for dk in range(3):          # kh = irow - o
                        irow = o + dk
                        for ci in range(3):
                            for co in range(3):
                                w[irow * 3 + ci, base + kw * 84 + o * 3 + co] = \
                                    inp["ll_lh_w"][i][co, ci, dk, kw] * m
            for o in range(out_r):
                row = LH_A[j] + i + 1 + o
                m = 1.0 if in_img(row) else 0.0
                for co in range(3):
                    b[o * 3 + co, j * 4 + i] = inp["ll_lh_b"][i][co] * m
    return w, b


def _pack_bias_m(bias, maskI, maskh1):
    """Masked bias for edge super-rows: col L*10+e = bias[:,colL]*maskI[:,j_e];
    cols 60..63 for h1 edge rows."""
    bm = np.zeros((128, 64), np.float32)
    for L in range(6):                      # 0=low0, 1..4=ll, 5=hl0
        for e, j in enumerate(EDGE):
            bm[:, L * 10 + e] = bias[:, L] * maskI[:, j]
    for t, m in enumerate(H1EDGE):
        bm[:, 60 + t] = bias[:, 6] * maskh1[:, m]
    return bm


# ---------------------------------------------------------------------------
# device program
# ---------------------------------------------------------------------------

# drain-engine rotation: 0=DVE, 1=Act (GPSIMD/Pool cannot read PSUM)
_PAT = [0, 1]


def _rot_drain(nc, rot, dst, acc, be, relu):
    eng = _PAT[rot[0] % len(_PAT)]
    rot[0] += 1
    if eng == 0:
        if relu:
            nc.vector.tensor_scalar(dst, acc, be, 0.0, ALU.add, ALU.max)
        else:
            nc.vector.tensor_scalar(dst, acc, be, None, ALU.add)
    elif eng == 1:
        nc.scalar.activation(dst, acc, AF.Relu if relu else AF.Identity,
                             bias=be)



def _emit_fold_layer(nc, psum, rot, src, dst, wts, lidx, bias_sb, bias_m_sb,
                     mask_sb, rng, relu):
    wd = [wts[:, k * 128:(k + 1) * 128] for k in range(3)]
    wq = [wts[:, 384 + k * 64:384 + (k + 1) * 64] for k in range(3)]
    for batch in _batches(*rng):
        accs = [psum.tile([128, 512], f32, name=f"acc{i}", tag=f"b{i}")
                for i in range(len(batch))]
        for kw in range(3):
            for i, j in enumerate(batch):
                nc.tensor.matmul(accs[i][:], wd[kw], src[:, j, kw:kw + 512],
                                 start=(kw == 0), stop=False)
        for kw in range(3):
            for i, j in enumerate(batch):
                nc.tensor.matmul(accs[i][0:64, :], wq[kw][64:128, :],
                                 src[64:128, j - 1, kw:kw + 512],
                                 start=False, stop=False, tile_position=(64, 0))
            for i, j in enumerate(batch):
                nc.tensor.matmul(accs[i][64:128, :], wq[kw][0:64, :],
                                 src[0:64, j + 1, kw:kw + 512],
                                 start=False, stop=(kw == 2), tile_position=(0, 64))
        for i, j in enumerate(batch):
            ddst = dst[:, j, 1:513]
            if j <= ZT or j >= ZB:
                e = EDGE.index(j)
                nc.scalar.activation(ddst, accs[i][:],
                                     AF.Relu if relu else AF.Identity,
                                     bias=bias_m_sb[:, lidx * 10 + e:lidx * 10 + e + 1],
                                     scale=mask_sb[:, j:j + 1])
            else:
                _rot_drain(nc, rot, ddst, accs[i][:],
                           bias_sb[:, lidx:lidx + 1], relu)


def _emit_lh_extract(nc, bufA, lhA):
    for j in range(3):
        cnt = LH_IN0[j] // 2
        s0 = LH_A[j] // 2
        ba = bufA[:]
        for ch in range(3):
            for par in range(2):
                sp = 61 + ch + 64 * par
                src = bass.AP(ba.tensor,
                              ba.offset + sp * ba.ap[0][0] + s0 * SLAB + 1,
                              [[ba.ap[0][0], 1], [SLAB, cnt], [1, 512]])
                dst = _ap(lhA, par * 3 + ch, j * 514 + 1, [[6, cnt], [1, 512]])
                nc.sync.dma_start(dst, src)


def _emit_lh_layer(nc, psum, lh_src, lh_dst, w_lh_sb, bias_lh_sb, i):
    for j in range(3):
        in_r = LH_IN0[j] - 2 * i
        out_r = in_r - 2
        acc = psum.tile([3 * out_r, 512], f32, name=f"lhacc{j}", tag=f"b{j}")
        base = ((j * 4 + i) * 3) * 84
        for kw in range(3):
            nc.tensor.matmul(
                acc[:],
                w_lh_sb[0:3 * in_r, base + kw * 84:base + kw * 84 + 3 * out_r],
                lh_src[0:3 * in_r, j * 514 + kw:j * 514 + kw + 512],
                start=(kw == 0), stop=(kw == 2))
        nc.scalar.activation(lh_dst[0:3 * out_r, j * 514 + 1:j * 514 + 513],
                             acc[:], AF.Tanh,
                             bias=bias_lh_sb[0:3 * out_r, j * 4 + i:j * 4 + i + 1])


def _emit_lh_scatter(nc, lhA, bufA):
    for j in range(3):
        out_r = LH_IN0[j] - 8
        cnt = out_r // 2
        s0 = (LH_A[j] + 4) // 2
        ba = bufA[:]
        for ch in range(3):
            for par in range(2):
                sp = 61 + ch + 64 * par
                dst = bass.AP(ba.tensor,
                              ba.offset + sp * ba.ap[0][0] + s0 * SLAB + 1,
                              [[ba.ap[0][0], 1], [SLAB, cnt], [1, 512]])
                src = _ap(lhA, par * 3 + ch, j * 514 + 1, [[6, cnt], [1, 512]])
                nc.sync.dma_start(dst, src)


def _emit_feats_half(nc, bufA, xT, yT, half):
    """Gather 16 owned super-rows of I4 lh into xT/yT [20, 16*SLAB] and
    multiply in place: xT <- xT * yT (the 10 quadratic features x2 par)."""
    s0 = OWN0 + half * 16
    XRUNS = [(0, 4), (4, 3), (7, 2)]
    YRUNS = [[0], [1, 4], [2, 5, 7]]
    ba = bufA[:]
    for par in range(2):
        for ci in range(3):
            sp = 61 + ci + 64 * par
            soff = ba.offset + sp * ba.ap[0][0] + s0 * SLAB
            p0, n = XRUNS[ci]
            src = bass.AP(ba.tensor, soff,
                          [[ba.ap[0][0], 1], [0, n], [1, 16 * SLAB]])
            dst = _ap(xT, 10 * par + p0, 0, [[1, n], [1, 16 * SLAB]])
            nc.sync.dma_start(dst, src)
            for yp in YRUNS[ci]:
                srcy = bass.AP(ba.tensor, soff,
                               [[ba.ap[0][0], 1], [1, 16 * SLAB]])
                dsty = _ap(yT, 10 * par + yp, 0, [[1, 1], [1, 16 * SLAB]])
                nc.sync.dma_start(dsty, srcy)
    nc.vector.tensor_mul(xT[:], xT[:], yT[:])


def _emit_tform_half(nc, psum, rot, wm20, xT, outstage, out_d, half):
    for sub in range(4):
        srs = [OWN0 + half * 16 + sub * 4 + i for i in range(4)]
        accs = [psum.tile([6, 512], f32, name=f"acc{i}", tag=f"b{i}")
                for i in range(len(srs))]
        for i, j in enumerate(srs):
            fo = (j - OWN0 - half * 16) * SLAB + 1
            nc.tensor.matmul(accs[i][:], wm20[:], xT[:, fo:fo + 512],
                             start=True, stop=True)
        for i, j in enumerate(srs):
            so = i * 512
            eng = _PAT[rot[0] % len(_PAT)]
            rot[0] += 1
            if eng == 0:
                nc.vector.tensor_copy(outstage[:, so:so + 512], accs[i][:])
            elif eng == 1:
                nc.scalar.activation(outstage[:, so:so + 512], accs[i][:],
                                     AF.Copy)

        r0 = 2 * (srs[0] - OWN0)
        for par in range(2):
            sap = _ap(outstage, par * 3, 0, [[1, 3], [512, 4], [1, 512]])
            dap = bass.AP(out_d[:].tensor, (r0 + par) * 512,
                          [[64 * 512, 3], [2 * 512, 4], [1, 512]])
            nc.sync.dma_start(dap, sap)


def _build_program(debug=False):
    nc = bacc.Bacc("TRN2", target_bir_lowering=False, debug=False,
                   num_devices=NCORES)

    x_in = nc.dram_tensor("x", [3, LR, SLAB], bf16, kind="ExternalInput")
    wmain_in = nc.dram_tensor("w_main", [128, 5 * 576 + 768], bf16,
                              kind="ExternalInput")
    wlow0_in = nc.dram_tensor("w_low0", [27, 64], bf16, kind="ExternalInput")
    wh23_in = nc.dram_tensor("w_h2h3", [64, 1152], bf16, kind="ExternalInput")
    wsel_in = nc.dram_tensor("w_sel", [30, 20], bf16, kind="ExternalInput")
    cmask_in = nc.dram_tensor("cmask", [30, 6], f32, kind="ExternalInput")
    pmask_in = nc.dram_tensor("pmask", [20, 6], f32, kind="ExternalInput")
    wlin_in = nc.dram_tensor("w_lin", [64, 30], f32, kind="ExternalInput")
    bias_in = nc.dram_tensor("bias", [128, 9], f32, kind="ExternalInput")
    biasm_in = nc.dram_tensor("bias_m", [128, 64], f32, kind="ExternalInput")
    linb_in = nc.dram_tensor("lin_b", [30, 1], f32, kind="ExternalInput")
    maskI_in = nc.dram_tensor("mask_i", [128, NSR], f32, kind="ExternalInput")
    maskh1_in = nc.dram_tensor("mask_h1", [128, 20], f32, kind="ExternalInput")
    wlh_in = nc.dram_tensor("w_lh", [90, 36 * 84], bf16, kind="ExternalInput")
    biaslh_in = nc.dram_tensor("bias_lh", [84, 12], f32, kind="ExternalInput")

    out_d = nc.dram_tensor("out", [3, 64, W], f32, kind="ExternalOutput")

    cc_in = nc.dram_tensor("cc_in", [64 * 4 * 32], bf16)
    cc_gath = nc.dram_tensor("cc_gath", [NCORES * 64 * 4 * 32], bf16,
                             addr_space="Shared")
    dbg = {}
    if debug:
        for nm, shp, dt in [("i0", [128, NSR * SLAB], bf16),
                            ("i4", [128, NSR * SLAB], bf16),
                            ("hl0", [128, NSR * SLAB], bf16),
                            ("lh", [90, 3 * 514], bf16),
                            ("h1", [128, 18 * 256], bf16),
                            ("pool1", [64, 20 * 130], bf16),
                            ("pool2", [64, 4 * 32], bf16),
                            ("wp", [30, 1], f32)]:
            dbg[nm] = nc.dram_tensor("dbg_" + nm, shp, dt, kind="ExternalOutput")

    rot = [0]
    with tile.TileContext(nc) as tc, ExitStack() as ctx:
        pers = ctx.enter_context(tc.tile_pool(name="pers", bufs=1))
        psum = ctx.enter_context(tc.tile_pool(name="psum", bufs=2, space="PSUM"))

        w_low0 = pers.tile([27, 64], bf16)
        nc.sync.dma_start(w_low0[:], wlow0_in[:])
        bias_sb = pers.tile([128, 9], f32)
        nc.sync.dma_start(bias_sb[:], bias_in[:])
        bias_m_sb = pers.tile([128, 64], f32)
        nc.sync.dma_start(bias_m_sb[:], biasm_in[:])
        mask_sb = pers.tile([128, NSR], f32)
        nc.sync.dma_start(mask_sb[:], maskI_in[:])
        w_main = pers.tile([128, 5 * 576 + 768], bf16)
        nc.sync.dma_start(w_main[:], wmain_in[:])
        w_lh = pers.tile([90, 36 * 84], bf16)
        nc.sync.dma_start(w_lh[:], wlh_in[:])
        bias_lh_sb = pers.tile([84, 12], f32)
        nc.sync.dma_start(bias_lh_sb[:], biaslh_in[:])
        w_h2h3 = pers.tile([64, 1152], bf16)
        nc.sync.dma_start(w_h2h3[:], wh23_in[:])
        w_sel = pers.tile([30, 20], bf16)
        nc.sync.dma_start(w_sel[:], wsel_in[:])
        cmask_sb = pers.tile([30, 6], f32)
        nc.sync.dma_start(cmask_sb[:], cmask_in[:])
        pmask_sb = pers.tile([20, 6], f32)
        nc.sync.dma_start(pmask_sb[:], pmask_in[:])
        w_lin = pers.tile([64, 30], f32)
        nc.sync.dma_start(w_lin[:], wlin_in[:])
        linb_sb = pers.tile([30, 1], f32)
        nc.sync.dma_start(linb_sb[:], linb_in[:])
        maskh1_sb = pers.tile([128, 20], f32)
        nc.sync.dma_start(maskh1_sb[:], maskh1_in[:])

        bufA = pers.tile([128, NSR, SLAB], bf16)
        bufB = pers.tile([128, NSR, SLAB], bf16)
        lhA = pers.tile([90, 3 * 514], bf16)
        lhB = pers.tile([90, 3 * 514], bf16)
        nc.gpsimd.memset(bufA[:], 0.0)
        nc.gpsimd.memset(bufB[:], 0.0)
        nc.gpsimd.memset(lhA[:], 0.0)
        nc.gpsimd.memset(lhB[:], 0.0)

        # ---- low0 via im2col: out rows 2..85 = slots 0..83, chunks of 22
        with tc.tile_pool(name="imcp", bufs=2) as imcp:
            for c0 in range(0, 84, 22):
                ns = min(22, 84 - c0)
                imc = imcp.tile([27, 22 * 512], bf16, name="imc", tag="imc")
                for kh in range(3):
                    for kw in range(3):
                        t = kh * 3 + kw
                        src = bass.AP(x_in[:].tensor,
                                      (c0 + 1 + kh) * SLAB + kw,
                                      [[LR * SLAB, 3], [SLAB, ns], [1, 512]])
                        nc.sync.dma_start(imc[3 * t:3 * t + 3, 0:ns * 512], src)
                srs = [j for j in range(R_I0[0], R_I0[1] + 1)
                       if c0 <= 2 * j - 2 and 2 * j - 1 < c0 + ns]
                for bt in _batches(srs[0], srs[-1]):
                    accs = [psum.tile([128, 512], f32, name=f"acc{i}",
                                      tag=f"b{i}") for i in range(len(bt))]
                    for i, j in enumerate(bt):
                        se = 2 * j - 2 - c0
                        nc.tensor.matmul(accs[i][0:64, :], w_low0[:],
                                         imc[:, se * 512:(se + 1) * 512],
                                         start=True, stop=True,
                                         tile_position=(0, 0))
                        nc.tensor.matmul(accs[i][64:128, :], w_low0[:],
                                         imc[:, (se + 1) * 512:(se + 2) * 512],
                                         start=True, stop=True,
                                         tile_position=(0, 64))
                    for i, j in enumerate(bt):
                        ddst = bufA[:, j, 1:513]
                        if j <= ZT or j >= ZB:
                            e = EDGE.index(j)
                            nc.scalar.activation(
                                ddst, accs[i][:], AF.Identity,
                                bias=bias_m_sb[:, e:e + 1],
                                scale=mask_sb[:, j:j + 1])
                        else:
                            _rot_drain(nc, rot, ddst, accs[i][:],
                                       bias_sb[:, 0:1], False)

        if debug:
            nc.sync.dma_start(dbg["i0"][:],
                              bufA[:].rearrange("p a b -> p (a b)"))

        # ---- lh extraction (I0 lh -> L layout) ----
        _emit_lh_extract(nc, bufA, lhA)

        with tc.tile_pool(name="tform", bufs=1) as tfp:
            xT0 = tfp.tile([20, 16 * SLAB], bf16)
            xT1 = tfp.tile([20, 16 * SLAB], bf16)
            yT = tfp.tile([20, 16 * SLAB], bf16)
            outstage = tfp.tile([6, 4 * 512], f32)
            wm20 = tfp.tile([20, 6], bf16)
            nc.gpsimd.memset(xT0[:], 1.0)
            nc.gpsimd.memset(xT1[:], 1.0)
            nc.gpsimd.memset(yT[:], 1.0)

            # ---- ll layers + hl0, lh layers interleaved ----
            bufs = [bufA, bufB]
            lhs = [lhA, lhB]
            for i in range(4):
                _emit_fold_layer(nc, psum, rot, bufs[i % 2], bufs[(i + 1) % 2],
                                 w_main[:, i * 576:(i + 1) * 576], 1 + i,
                                 bias_sb, bias_m_sb, mask_sb, R_LL[i], True)
                _emit_lh_layer(nc, psum, lhs[i % 2], lhs[(i + 1) % 2],
                               w_lh, bias_lh_sb, i)
            _emit_fold_layer(nc, psum, rot, bufA, bufB, w_main[:, 4 * 576:5 * 576],
                             5, bias_sb, bias_m_sb, mask_sb, R_HL0, False)
            # I4 lh lives in lhA after 4 layers; scatter into bufA slab
            _emit_lh_scatter(nc, lhA, bufA)
            if debug:
                nc.sync.dma_start(dbg["i4"][:],
                                  bufA[:].rearrange("p a b -> p (a b)"))
                nc.sync.dma_start(dbg["hl0"][:],
                                  bufB[:].rearrange("p a b -> p (a b)"))
                nc.sync.dma_start(dbg["lh"][:], lhA[:])

            with tc.tile_pool(name="hlp", bufs=1) as hlp:
                pool1 = hlp.tile([64, 20, 130], bf16)
                nc.gpsimd.memset(pool1[:], 0.0)
                p2f = hlp.tile([64, 34, 34], bf16)
                nc.gpsimd.memset(p2f[:], 0.0)

                # ---- h1 (stride-2 fold conv from bufB) ----
                wh1 = w_main[:, 5 * 576:]
                wA = [wh1[:, k * 128:(k + 1) * 128] for k in range(3)]
                wB = [wh1[:, 384 + k * 64:384 + (k + 1) * 64] for k in range(3)]
                wC = [wh1[:, 576 + k * 64:576 + (k + 1) * 64] for k in range(3)]
                with tc.tile_pool(name="h1p", bufs=1) as h1p:
                    h1fold = h1p.tile([128, 18, 256], bf16)
                    for batch in _batches(*R_H1):
                        accs = [psum.tile([128, 256], f32, name=f"acc{i}",
                                          tag=f"b{i}") for i in range(len(batch))]
                        for kw in range(3):
                            for i, m in enumerate(batch):
                                nc.tensor.matmul(accs[i][:], wA[kw],
                                                 bufB[:, 2 * m, kw:kw + 512:2],
                                                 start=(kw == 0), stop=False)
                        for kw in range(3):
                            for i, m in enumerate(batch):
                                nc.tensor.matmul(accs[i][0:64, :], wB[kw][64:128, :],
                                                 bufB[64:128, 2 * m - 1, kw:kw + 512:2],
                                                 start=False, stop=False,
                                                 tile_position=(64, 0))
                            for i, m in enumerate(batch):
                                nc.tensor.matmul(accs[i][64:128, :], wC[kw][:],
                                                 bufB[:, 2 * m + 1, kw:kw + 512:2],
                                                 start=False, stop=(kw == 2),
                                                 tile_position=(0, 64))
                        for i, m in enumerate(batch):
                            sl = h1fold[:, m - 2, :]
                            if m in H1EDGE:
                                t = H1EDGE.index(m)
                                nc.scalar.activation(
                                    sl, accs[i][:], AF.Relu,
                                    bias=bias_m_sb[:, 60 + t:60 + t + 1],
                                    scale=maskh1_sb[:, m:m + 1])
                            else:
                                _rot_drain(nc, rot, sl, accs[i][:],
                                           bias_sb[:, 6:7], True)
                    if debug:
                        nc.sync.dma_start(
                            dbg["h1"][:],
                            h1fold[:].rearrange("p a b -> p (a b)"))

                    # pool1[ch, 2+m', 1:129] = max over (par, colpair) of h1fold
                    t1 = h1p.tile([128, 18, 128], bf16)
                    nc.vector.tensor_max(t1[:], h1fold[:, :, 0:256:2],
                                         h1fold[:, :, 1:256:2])
                    # engines can't cross partition starts; realign via DMA
                    t1b = h1p.tile([64, 18, 128], bf16)
                    nc.sync.dma_start(t1b[:], t1[64:128, :, :])
                    nc.vector.tensor_max(pool1[:, 2:20, 1:129],
                                         t1[0:64, :, :], t1b[:])
                if debug:
                    nc.sync.dma_start(dbg["pool1"][:],
                                      pool1[:].rearrange("p a b -> p (a b)"))

                # ---- h2 via im2col (DVE gather, 9 taps, K=64) ----
                with tc.tile_pool(name="h2p", bufs=1) as h2p:
                    imc2 = h2p.tile([64, 9 * 512], bf16)
                    for t in range(9):
                        kh, kw = divmod(t, 3)
                        src = _ap(pool1, 0, (2 + kh) * 130 + kw,
                                  [[1, 64], [2 * 130, 8], [2, 64]])
                        nc.vector.tensor_copy(imc2[:, t * 512:(t + 1) * 512], src)
                    acc2 = psum.tile([64, 512], f32, name="acc0", tag="b0")
                    for t in range(9):
                        nc.tensor.matmul(acc2[:], w_h2h3[:, t * 64:(t + 1) * 64],
                                         imc2[:, t * 512:(t + 1) * 512],
                                         start=(t == 0), stop=(t == 8))
                    h2sb = h2p.tile([64, 8, 64], bf16)
                    nc.scalar.activation(h2sb[:].rearrange("p a b -> p (a b)"),
                                         acc2[:], AF.Relu, bias=bias_sb[0:64, 7:8])

                    # pool2 -> cc_in
                    tmp2 = h2p.tile([64, 8, 32], bf16)
                    nc.vector.tensor_max(tmp2[:], h2sb[:, :, 0:64:2],
                                         h2sb[:, :, 1:64:2])
                    pool2 = h2p.tile([64, 4, 32], bf16)
                    nc.vector.tensor_max(pool2[:], tmp2[:, 0:8:2, :],
                                         tmp2[:, 1:8:2, :])
                    nc.sync.dma_start(cc_in[:],
                                      pool2[:].rearrange("p a b -> p (a b)"))
                    if debug:
                        nc.sync.dma_start(dbg["pool2"][:],
                                          pool2[:].rearrange("p a b -> p (a b)"))

                # feats half 0 (independent of the collective; fills the
                # h1/h2 window)
                _emit_feats_half(nc, bufA, xT0, yT, 0)

                # ---- AllGather pool2 ----
                with tc.tile_critical():
                    cc_sem = nc.alloc_semaphore("cc_sem")
                    nc.gpsimd.collective_compute(
                        "AllGather", ALU.bypass,
                        replica_groups=[list(range(NCORES))],
                        ins=[cc_in[:]], outs=[cc_gath[:]],
                    ).then_inc(cc_sem)
                    nc.gpsimd.wait_ge(cc_sem, 1)

                # feats half 1 overlaps the collective
                _emit_feats_half(nc, bufA, xT1, yT, 1)

                # ---- h3 tail (replicated) ----
                with tc.tile_pool(name="h3p", bufs=1) as h3p:
                    for q in range(NCORES):
                        src = bass.AP(cc_gath[:].tensor, q * 64 * 4 * 32,
                                      [[4 * 32, 64], [32, 4], [1, 32]])
                        nc.sync.dma_start(p2f[:, 1 + 4 * q:5 + 4 * q, 1:33], src)
                    imc3 = h3p.tile([64, 9 * 256], bf16)
                    for t in range(9):
                        kh, kw = divmod(t, 3)
                        src = _ap(p2f, 0, kh * 34 + kw,
                                  [[1, 64], [2 * 34, 16], [2, 16]])
                        nc.vector.tensor_copy(imc3[:, t * 256:(t + 1) * 256], src)
                    acc3 = psum.tile([64, 256], f32, name="acc1", tag="b1")
                    for t in range(9):
                        nc.tensor.matmul(acc3[:],
                                         w_h2h3[:, 576 + t * 64:576 + (t + 1) * 64],
                                         imc3[:, t * 256:(t + 1) * 256],
                                         start=(t == 0), stop=(t == 8))
                    h3sb = h3p.tile([64, 16, 16], bf16)
                    nc.scalar.activation(h3sb[:].rearrange("p a b -> p (a b)"),
                                         acc3[:], AF.Relu, bias=bias_sb[0:64, 8:9])
                    tmp3 = h3p.tile([64, 16, 8], bf16)
                    nc.vector.tensor_max(tmp3[:], h3sb[:, :, 0:16:2],
                                         h3sb[:, :, 1:16:2])
                    h3pool = h3p.tile([64, 8, 8], f32)
                    nc.vector.tensor_max(h3pool[:], tmp3[:, 0:16:2, :],
                                         tmp3[:, 1:16:2, :])
                    gsum = h3p.tile([64, 1], f32)
                    nc.vector.reduce_sum(gsum[:],
                                         h3pool[:].rearrange("p a b -> p (a b)"),
                                         axis=mybir.AxisListType.X)
                    accW = psum.tile([30, 1], f32, name="acc2", tag="b2")
                    nc.tensor.matmul(accW[:], w_lin[:], gsum[:],
                                     start=True, stop=True)
                    wp_sb = h3p.tile([30, 1], f32)
                    nc.scalar.activation(wp_sb[:], accW[:], AF.Identity,
                                         bias=linb_sb[:])
                    if debug:
                        nc.sync.dma_start(dbg["wp"][:], wp_sb[:])
                    wpR = h3p.tile([30, 6], bf16)
                    nc.vector.tensor_scalar_mul(wpR[:], cmask_sb[:], wp_sb[:])
                    accM = psum.tile([20, 6], f32, name="acc3", tag="b3")
                    nc.tensor.matmul(accM[:], w_sel[:], wpR[:],
                                     start=True, stop=True)
                    nc.vector.tensor_tensor(wm20[:], accM[:], pmask_sb[:],
                                            ALU.mult)

            # ---- Tform ----
            _emit_tform_half(nc, psum, rot, wm20, xT0, outstage, out_d, 0)
            _emit_tform_half(nc, psum, rot, wm20, xT1, outstage, out_d, 1)

    nc.compile()
    return nc


# ---------------------------------------------------------------------------
# host entry
# ---------------------------------------------------------------------------

def kernel(**inputs):
    inp = {k: np.asarray(v) for k, v in inputs.items()}
    debug = bool(_cached.get("debug", False))
    key = ("nc", debug)
    if key not in _cached:
        _cached[key] = _build_program(debug=debug)
    nc = _cached[key]

    pk = _pack_weights(inp)
    x = np.asarray(inp["x"], np.float32)[0]

    shared = {
        "w_main": pk["w_main"].astype(nbf),
        "w_low0": pk["w_low0"].astype(nbf),
        "w_h2h3": pk["w_h2h3"].astype(nbf),
        "w_sel": pk["w_sel"].astype(nbf),
        "cmask": pk["cmask"],
        "pmask": pk["pmask"],
        "w_lin": pk["w_lin"],
        "bias": pk["bias"],
        "lin_b": pk["lin_b"],
    }
    in_maps = []
    par_col = (np.arange(128) // 64)[:, None]
    for c in range(NCORES):
        g0 = 64 * c - HALO
        xs = np.zeros((3, LR, SLAB), np.float32)
        lo, hi = max(0, -g0), min(LR, H - g0)
        xs[:, lo:hi, 1:513] = x[:, g0 + lo:g0 + hi, :]
        gI = g0 + 2 * np.arange(NSR)[None, :] + par_col
        maskI = ((gI >= 0) & (gI < H)).astype(np.float32)
        gh = 32 * c - 6 + 2 * np.arange(20)[None, :] + par_col
        maskh1 = ((gh >= 0) & (gh < 256)).astype(np.float32)
        w_lh_c, bias_lh_c = _pack_lh_core(inp, c)
        im = dict(shared)
        im["x"] = xs.astype(nbf)
        im["mask_i"] = maskI
        im["mask_h1"] = maskh1
        im["bias_m"] = _pack_bias_m(pk["bias"], maskI, maskh1)
        im["w_lh"] = w_lh_c.astype(nbf)
        im["bias_lh"] = bias_lh_c
        in_maps.append(im)

    res = run_bass_kernel_spmd(nc, in_maps, list(range(NCORES)))
    _cached["last_results"] = res
    out = np.concatenate([res.results[c]["out"] for c in range(NCORES)], axis=1)
    return out[None].astype(np.float32)
